# revision 2
# baseline (speedup 1.0000x reference)
"""Trainium2 Bass kernel for nn_ActionDecoder (MoE-routed 2-layer GELU MLP).

Problem: per batch row b (2048 rows x 16 timesteps), route through the
embodiment_ids[b]-th expert MLP: out = GELU(x @ W1[e] + b1[e]) @ W2[e] + b2[e].
x: [2048, 16, 512] f32, W1: [4, 512, 1024], W2: [4, 1024, 28].

Strategy (expert-parallel): host sorts batch rows by embodiment, gives each of
the 8 cores one expert (2 cores per expert, half the expert's rows each). Each
core runs a dense 2-layer MLP over its tokens with its own expert's weights
(weights are per-core *data*, so one SPMD program serves all cores).

Layer 1 runs in fp8(e4m3) DoubleRow mode: each DR matmul streams two
128-contraction chunks at 0.5 cycles/output-column (4x fp16 MAC rate). To hit
fp16-level accuracy, x and W1 are split hi+lo at a SHARED power-of-2 scale
(e4m3 subnormals absorb the tiny lo parts), and h is assembled from three
product groups -- xh@W1h + xh@W1l + xl@W1h -- all accumulating in one fp32
PSUM at the common scale 2^-16, which the GELU activation's `scale` undoes.
6 DR instructions replace 4 fp16 matmuls per h-chunk: 1.33x on the PE.
Layer 2 (M=28) stays fp16, packed 4-wide into PE column groups.

Perf notes:
- Token dim tiled as 512-token tiles plus one 128-multiple remainder tile so
  SPMD padding is at most 127 tokens.
- A few dependency-free warmup matmuls run during the initial DMA wait to
  lift the PE HAM clock gate to 8/8 before real work arrives.
"""

import os

import numpy as np
import ml_dtypes

import concourse.bacc as bacc
import concourse.mybir as mybir
from concourse.tile import TileContext
from concourse.bass_utils import run_bass_kernel_spmd

# Model dims (hardcoded per problem spec)
D = 512      # d_model
H = 1024     # hidden
A = 28       # max action dim
E = 4        # n embodiments
N_CORES = 8
P = 128      # partitions
TILE = 512   # main token tile
GRAIN = 128  # token granularity (min tile)
KC = D // P  # 4 contraction chunks for layer 1
HC = H // P  # 8 hidden chunks

S_X = 32.0    # fp8 scale for x (max|x|*32 ~ 176 < 240)
S_W = 2048.0  # fp8 scale for W1 (max|W1|*2048 ~ 192 < 240)
DESCALE = 1.0 / (S_X * S_W)

PS_H_BUFS = 3      # fused-gelu L1 PSUM slots (2 banks each)
PS_O_BUFS = 2      # layer-2 PSUM slots (1 bank each); ps_h*2 + ps_o <= 8
N_WARMUP_MM = 20   # spans the DMA head so HAM is at 8/8 when real work arrives
PACK_L2 = True     # pack layer-2 into PE column groups

F32 = mybir.dt.float32
F16 = mybir.dt.float16
F8 = mybir.dt.float8e4
NP_F8 = ml_dtypes.float8_e4m3
DR = mybir.MatmulPerfMode.DoubleRow

_PROGRAM_CACHE = {}

# Set by test harness to collect a profile: None | dict (filled with results)
TRACE_SINK = None


def _tile_sizes(ntok):
    sizes = [TILE] * (ntok // TILE)
    if ntok % TILE:
        sizes.append(ntok % TILE)  # remainder last: short pipeline tail
    return sizes


def _build_program(ntok, fuse_gelu):
    assert ntok % GRAIN == 0
    sizes = _tile_sizes(ntok)
    nc = bacc.Bacc()

    # x hi/lo are tile-blocked: tile t occupies columns [KC*off, KC*(off+size))
    # as a [KC, size] block, so every DMA reads contiguous per-partition runs
    xh_in = nc.declare_dram_parameter("xh", [P, KC * ntok], F8, isOutput=False)
    xl_in = nc.declare_dram_parameter("xl", [P, KC * ntok], F8, isOutput=False)
    w1h_in = nc.declare_dram_parameter("w1h", [P, HC, KC, P], F8, isOutput=False)
    w1l_in = nc.declare_dram_parameter("w1l", [P, HC, KC, P], F8, isOutput=False)
    w2_in = nc.declare_dram_parameter("w2", [P, HC, A], F16, isOutput=False)
    b1_in = nc.declare_dram_parameter("b1", [P, HC], F32, isOutput=False)
    b2_in = nc.declare_dram_parameter("b2", [A, 1], F32, isOutput=False)
    out = nc.declare_dram_parameter("out", [A, ntok], F32, isOutput=True)

    with TileContext(nc) as tc:
        with (
            tc.tile_pool(name="wpool", bufs=1) as wpool,
            tc.tile_pool(name="xpool", bufs=4) as xpool,
            tc.tile_pool(name="hpool", bufs=3) as hpool,
            tc.tile_pool(name="opool", bufs=3) as opool,
            tc.tile_pool(name="ps_h", bufs=PS_H_BUFS if fuse_gelu else 6, space="PSUM") as ps_h_pool,
            tc.tile_pool(name="ps_o", bufs=PS_O_BUFS, space="PSUM") as ps_o_pool,
        ):
            # --- PE warmup: no data deps, runs during the initial DMA wait ---
            if N_WARMUP_MM:
                warm_x = wpool.tile([P, TILE], F16)
                nc.gpsimd.memset(warm_x, 0.0)
                warm_shape = [P, 2, TILE] if fuse_gelu else [P, TILE]
                warm_ps = ps_h_pool.tile(warm_shape, F32, tag="ps_h")
                warm_ps = warm_ps[:, 0] if fuse_gelu else warm_ps
                for _ in range(N_WARMUP_MM):
                    nc.tensor.matmul(warm_ps, warm_x[:, :P], warm_x,
                                     start=True, stop=True)

            # --- Weight/x loads, interleaved pieces so the first matmuls
            # start as early as possible. xh0 + w1h are the critical path;
            # xl0 + w1l stream behind them on other queues. ---
            w1h_sb = wpool.tile([P, HC, KC, P], F8)
            w1l_sb = wpool.tile([P, HC, KC, P], F8)
            xh_sb0 = xpool.tile([P, KC, sizes[0]], F8, tag="x")
            xl_sb0 = xpool.tile([P, KC, sizes[0]], F8, tag="x")
            b1_sb = wpool.tile([P, HC], F32)
            b2_sb = wpool.tile([A, 1], F32)
            nc.scalar.dma_start(
                out=xh_sb0,
                in_=xh_in[:, 0:KC * sizes[0]].rearrange("p (kc n) -> p kc n", kc=KC))
            nc.sync.dma_start(out=w1h_sb[:, 0:HC // 2], in_=w1h_in[:, 0:HC // 2])
            nc.sync.dma_start(out=w1h_sb[:, HC // 2:], in_=w1h_in[:, HC // 2:])
            nc.scalar.dma_start(
                out=xl_sb0,
                in_=xl_in[:, 0:KC * sizes[0]].rearrange("p (kc n) -> p kc n", kc=KC))
            nc.sync.dma_start(out=w1l_sb[:, 0:HC // 2], in_=w1l_in[:, 0:HC // 2])
            nc.sync.dma_start(out=w1l_sb[:, HC // 2:], in_=w1l_in[:, HC // 2:])
            nc.gpsimd.dma_start(out=b1_sb, in_=b1_in[:])
            nc.gpsimd.dma_start(out=b2_sb, in_=b2_in[:])
            w2_sb = wpool.tile([P, HC, A], F16)
            nc.gpsimd.dma_start(out=w2_sb, in_=w2_in[:])
            xh_sb1 = xl_sb1 = None
            if len(sizes) > 1:
                xh_sb1 = xpool.tile([P, KC, sizes[1]], F8, tag="x")
                xl_sb1 = xpool.tile([P, KC, sizes[1]], F8, tag="x")
                a = KC * sizes[0]
                nc.gpsimd.dma_start(
                    out=xh_sb1,
                    in_=xh_in[:, a:a + KC * sizes[1]].rearrange("p (kc n) -> p kc n", kc=KC))
                nc.gpsimd.dma_start(
                    out=xl_sb1,
                    in_=xl_in[:, a:a + KC * sizes[1]].rearrange("p (kc n) -> p kc n", kc=KC))

            def emit_l1_mms(ps, hc, xh_sb, xl_sb):
                """3-term fp8 DoubleRow: ps = (xh@W1h + xh@W1l + xl@W1h)."""
                groups = ((w1h_sb, xh_sb), (w1l_sb, xh_sb), (w1h_sb, xl_sb))
                n = len(groups) * 2
                i = 0
                for w_sb, x_sb in groups:
                    for kc in (0, 2):
                        nc.tensor.matmul(
                            ps,
                            w_sb[:, hc, kc:kc + 2],
                            x_sb[:, kc:kc + 2],
                            start=(i == 0),
                            stop=(i == n - 1),
                            perf_mode=DR,
                        )
                        i += 1

            def emit_l2(h_sb, off, size, t, packed):
                """Layer 2: out[:, off:off+size] = W2^T h + b2."""
                o_sb = opool.tile([A, size], F32, tag="o")
                if packed:
                    # 4 h-chunks run concurrently in the 4 PE column groups,
                    # accumulating 2 rounds; strips combined on DVE (which may
                    # read at most one PSUM operand per instruction).
                    o_ps = ps_o_pool.tile([P, size], F32, tag="ps_o")
                    for r in range(2):
                        for j in range(4):
                            hc = r * 4 + j
                            nc.tensor.matmul(
                                o_ps[32 * j:32 * j + A, :],
                                w2_sb[:, hc],
                                h_sb[:, hc],
                                start=(r == 0),
                                stop=(r == 1),
                                tile_position=(0, 32 * j),
                            )
                    nc.vector.tensor_scalar_add(o_sb, o_ps[0:A], b2_sb)
                    nc.vector.tensor_add(o_sb, o_sb, o_ps[32:32 + A])
                    nc.vector.tensor_add(o_sb, o_sb, o_ps[64:64 + A])
                    nc.vector.tensor_add(o_sb, o_sb, o_ps[96:96 + A])
                else:
                    o_ps = ps_o_pool.tile([A, size], F32, tag="ps_o")
                    for hc in range(HC):
                        nc.tensor.matmul(
                            o_ps,
                            w2_sb[:, hc],
                            h_sb[:, hc],
                            start=(hc == 0),
                            stop=(hc == HC - 1),
                        )
                    if fuse_gelu:
                        # b2 == 0: PSUM->SBUF copy on ACT (idle at the tail)
                        # so the store doesn't queue behind the previous
                        # tile's DVE strip-combine on the in-order Vector
                        nc.scalar.activation(o_sb, o_ps,
                                             mybir.ActivationFunctionType.Copy)
                    else:
                        nc.vector.tensor_scalar_add(o_sb, o_ps, b2_sb)
                # alternate store queues so the final two stores issue in
                # parallel instead of serializing on one engine
                eng = nc.sync if t % 2 == 0 else nc.scalar
                eng.dma_start(out=out[:, off:off + size], in_=o_sb)

            # Layer 2 for tile t is emitted mid-way through layer 1 of tile
            # t+1 so its matmuls never wait on a just-finished GELU (PE is
            # in-order) and its DVE/store epilogue drains under compute. The
            # final tile uses unpacked L2: its single-op DVE epilogue keeps
            # the drain tail short.
            pend = None
            off = 0
            for t, size in enumerate(sizes):
                if t == 0:
                    xh_sb, xl_sb = xh_sb0, xl_sb0
                elif t == 1 and xh_sb1 is not None:
                    xh_sb, xl_sb = xh_sb1, xl_sb1
                else:
                    xh_sb = xpool.tile([P, KC, size], F8, tag="x")
                    xl_sb = xpool.tile([P, KC, size], F8, tag="x")
                    a = KC * off
                    nc.sync.dma_start(
                        out=xh_sb,
                        in_=xh_in[:, a:a + KC * size].rearrange("p (kc n) -> p kc n", kc=KC))
                    nc.sync.dma_start(
                        out=xl_sb,
                        in_=xl_in[:, a:a + KC * size].rearrange("p (kc n) -> p kc n", kc=KC))

                # --- Layer 1: h = gelu((xh@W1h + xh@W1l + xl@W1h)*2^-16 + b1)
                h_sb = hpool.tile([P, HC, size], F16, tag="h")

                def flush_pend(pend=pend):
                    if pend is not None:
                        packed = PACK_L2 and pend[3] < len(sizes) - 1
                        emit_l2(*pend, packed)

                if fuse_gelu:
                    # b1 == 0: one ACTIVATE per pair of h-chunks (2 PSUM
                    # banks) halves ACT per-op overhead; ACT is otherwise
                    # rate-matched with PE and every hiccup stalls it.
                    for hg in range(HC // 2):
                        ps = ps_h_pool.tile([P, 2, size], F32, tag="ps_h")
                        for half in range(2):
                            emit_l1_mms(ps[:, half], hg * 2 + half, xh_sb, xl_sb)
                        nc.scalar.activation(
                            h_sb[:, hg * 2:hg * 2 + 2], ps,
                            mybir.ActivationFunctionType.Gelu,
                            scale=DESCALE,
                        )
                        if hg == 0:
                            # previous tile's layer 2 goes here: mid-tile so
                            # its DVE/store epilogue drains before this
                            # tile's L1 ends (shorter pipeline tail)
                            flush_pend()
                else:
                    for hc in range(HC):
                        ps = ps_h_pool.tile([P, size], F32, tag="ps_h")
                        emit_l1_mms(ps, hc, xh_sb, xl_sb)
                        nc.scalar.activation(
                            h_sb[:, hc], ps,
                            mybir.ActivationFunctionType.Gelu,
                            bias=b1_sb[:, hc:hc + 1],
                            scale=DESCALE,
                        )
                        if hc == 1:
                            flush_pend()

                pend = (h_sb, off, size, t)
                off += size

            packed = PACK_L2 and pend[3] < len(sizes) - 1
            emit_l2(*pend, packed)

    nc.finalize()
    return nc


def _q8(a, s):
    """Quantize a*s to e4m3 (round-to-nearest-even via ml_dtypes)."""
    return (a * s).astype(NP_F8)


def kernel(pred_action_latents, W1, b1, W2, b2, embodiment_ids):
    x = np.asarray(pred_action_latents, dtype=np.float32)
    W1 = np.asarray(W1, dtype=np.float32)
    b1 = np.asarray(b1)
    W2 = np.asarray(W2)
    b2 = np.asarray(b2)
    ids = np.asarray(embodiment_ids)

    B, T, _ = x.shape
    assert W1.shape[0] == E and N_CORES == 2 * E

    # --- Host-side routing/sharding ---
    order = np.argsort(ids, kind="stable")
    counts = np.bincount(ids, minlength=E)
    starts = np.concatenate([[0], np.cumsum(counts)])

    # core 2e, 2e+1 handle expert e (first/second half of its rows)
    core_rows = []
    for e in range(E):
        rows_e = order[starts[e]:starts[e + 1]]
        h1 = (len(rows_e) + 1) // 2
        core_rows.append(rows_e[:h1])
        core_rows.append(rows_e[h1:])

    max_tok = max(len(r) * T for r in core_rows)
    ntok = max(GRAIN, ((max_tok + GRAIN - 1) // GRAIN) * GRAIN)

    fuse_gelu = not np.any(b1) and not np.any(b2)
    key = (ntok, fuse_gelu)
    if key not in _PROGRAM_CACHE:
        _PROGRAM_CACHE[key] = _build_program(ntok, fuse_gelu)
    nc = _PROGRAM_CACHE[key]

    # hi/lo fp8 split of per-expert W1 at the shared scale S_W
    w1h_q = _q8(W1, S_W)                                   # [E, D, H] fp8
    w1l_q = _q8(W1 - w1h_q.astype(np.float32) / S_W, S_W)  # residual, same scale

    def _blocked_x(xr8, ntok):
        # tile-blocked [P, KC*ntok]: tile block t = [P, KC, size] with
        # (p, kc, n) = xr[off+n, kc*P+p]; contiguous per-partition runs
        blocks = []
        o = 0
        for size in _tile_sizes(ntok):
            blocks.append(
                xr8[o:o + size].reshape(size, KC, P).transpose(2, 1, 0).reshape(P, KC * size))
            o += size
        return np.ascontiguousarray(np.concatenate(blocks, axis=1))

    in_maps = []
    for c in range(N_CORES):
        e = c // 2
        rows = core_rows[c]
        ntok_real = len(rows) * T
        xr = np.zeros((ntok, D), dtype=np.float32)
        xr[:ntok_real] = x[rows].reshape(ntok_real, D)
        xh8 = _q8(xr, S_X)
        xl8 = _q8(xr - xh8.astype(np.float32) / S_X, S_X)
        # [P, HC, KC, 128]: (p, hc, kc, j) = W1[e, kc*P+p, hc*P+j]
        w1h_dev = np.ascontiguousarray(
            w1h_q[e].reshape(KC, P, HC, P).transpose(1, 2, 0, 3))
        w1l_dev = np.ascontiguousarray(
            w1l_q[e].reshape(KC, P, HC, P).transpose(1, 2, 0, 3))
        w2_dev = np.ascontiguousarray(
            W2[e].reshape(HC, P, A).transpose(1, 0, 2)
        ).astype(np.float16)
        b1_dev = np.ascontiguousarray(b1[e].reshape(HC, P).T).astype(np.float32)
        b2_dev = np.ascontiguousarray(b2[e].reshape(A, 1)).astype(np.float32)
        in_maps.append({
            "xh": _blocked_x(xh8, ntok), "xl": _blocked_x(xl8, ntok),
            "w1h": w1h_dev, "w1l": w1l_dev, "w2": w2_dev,
            "b1": b1_dev, "b2": b2_dev,
        })

    trace = TRACE_SINK is not None
    if trace:
        os.environ.pop("BASS_NEVER_TRACE", None)
    else:
        # An ambient BASS_TRACE would route run_bass_kernel_spmd through the
        # axon NTFF hook, which needs antenv.axon_hooks (absent in fresh
        # containers) — force tracing off unless explicitly requested.
        os.environ["BASS_NEVER_TRACE"] = "1"
    res = run_bass_kernel_spmd(nc, in_maps, core_ids=list(range(N_CORES)),
                               trace=trace)
    if trace:
        TRACE_SINK["exec_time_ns"] = res.exec_time_ns
        TRACE_SINK["mean_exec_time_ns"] = res.mean_exec_time_ns
        TRACE_SINK["profile_json"] = res.profile_json

    # --- Host-side unshard ---
    out_full = np.zeros((B, T, A), dtype=np.float32)
    for c in range(N_CORES):
        rows = core_rows[c]
        if len(rows) == 0:
            continue
        o = np.asarray(res.results[c]["out"])  # [A, ntok] f32
        out_full[rows] = o[:, :len(rows) * T].T.reshape(len(rows), T, A)
    return out_full


# revision 4
# speedup vs baseline: 1.3803x; 1.3803x over previous
"""Trainium2 Bass kernel for nn_ActionDecoder (MoE-routed 2-layer GELU MLP).

Problem: per batch row b (2048 rows x 16 timesteps), route through the
embodiment_ids[b]-th expert MLP: out = GELU(x @ W1[e] + b1[e]) @ W2[e] + b2[e].
x: [2048, 16, 512] f32, W1: [4, 512, 1024], W2: [4, 1024, 28].

Strategy (expert-parallel): host sorts batch rows by embodiment, gives each of
the 8 cores one expert (2 cores per expert, half the expert's rows each). Each
core runs a dense 2-layer MLP over its tokens with its own expert's weights
(weights are per-core *data*, so one SPMD program serves all cores). Activations
are fed transposed ([d, tok]) so both matmuls keep weights stationary; compute
in fp16 (same PE rate as bf16, 8x finer mantissa) with fp32 PSUM accumulation.
(fp8 was evaluated and rejected: e4m3 DoubleRow runs at the same column rate
as fp16 so accuracy-preserving hi/lo splits cancel the MAC gain, and
single-fp8 error (>2e-2 rel even with optimal linear corrections) fails the
gate; e3m4 DoubleRow is rejected by walrus codegen.)

Perf notes:
- Token dim tiled as one 256-token lead tile (so the first matmuls wait on
  only 256 KB of x), then 512-token tiles, then a 256..767 remainder; every
  size >= 256 keeps the PE at its 216 ns/matmul streaming rate.
- W1 is DMA'd in four 256 KB pieces so tile 0's first h-chunks start after
  one piece lands instead of after the full 1 MB.
- Layer 2 (M=28) packs 4 h-chunks into the 4 PE column groups concurrently
  (tile_position), then combines the 4 PSUM partition strips on DVE.
- A few dependency-free warmup matmuls run during the initial DMA wait to
  lift the PE HAM clock gate to 8/8 before real work arrives.
"""

import os

import numpy as np

import concourse.bacc as bacc
import concourse.mybir as mybir
from concourse.tile import TileContext
from concourse.bass_utils import run_bass_kernel_spmd

# Model dims (hardcoded per problem spec)
D = 512      # d_model
H = 1024     # hidden
A = 28       # max action dim
E = 4        # n embodiments
N_CORES = 8
P = 128      # partitions
TILE = 512   # main token tile
LEAD = 256   # first tile: small so compute starts on 256KB of x
GRAIN = 128  # token granularity (min tile)
KC = D // P  # 4 contraction chunks for layer 1
HC = H // P  # 8 hidden chunks

PS_H_BUFS = 3      # fused-gelu L1 PSUM slots (2 banks each)
PS_O_BUFS = 2      # layer-2 PSUM slots (1 bank each); ps_h*2 + ps_o <= 8
N_WARMUP_MM = 10   # cover the ~3us from clock start to first real matmul
PACK_L2 = True     # pack layer-2 into PE column groups

F32 = mybir.dt.float32
F16 = mybir.dt.float16

_PROGRAM_CACHE = {}

# Set by test harness to collect a profile: None | dict (filled with results)
TRACE_SINK = None


def _tile_sizes(ntok):
    if ntok <= TILE + LEAD:
        # too small to split a lead tile off; mirror the simple scheme
        sizes = [TILE] * (ntok // TILE)
        if ntok % TILE:
            sizes.append(ntok % TILE)
        return sizes
    rest = ntok - LEAD
    n_full = (rest - LEAD) // TILE  # keep remainder >= LEAD
    rem = rest - n_full * TILE      # in [LEAD, TILE+LEAD)
    return [LEAD] + [TILE] * n_full + [rem]


def _build_program(ntok, fuse_gelu):
    assert ntok % GRAIN == 0
    sizes = _tile_sizes(ntok)
    nc = bacc.Bacc()

    # x is tile-blocked: tile t occupies columns [KC*off, KC*(off+size)) as
    # a [KC, size] block, so every DMA reads 4KB-contiguous per-partition runs
    x_in = nc.declare_dram_parameter("x", [P, KC * ntok], F16, isOutput=False)
    w1_in = nc.declare_dram_parameter("w1", [P, HC, KC, P], F16, isOutput=False)
    w2_in = nc.declare_dram_parameter("w2", [P, HC, A], F16, isOutput=False)
    b1_in = nc.declare_dram_parameter("b1", [P, HC], F32, isOutput=False)
    b2_in = nc.declare_dram_parameter("b2", [A, 1], F32, isOutput=False)
    out = nc.declare_dram_parameter("out", [A, ntok], F32, isOutput=True)

    with TileContext(nc) as tc:
        with (
            tc.tile_pool(name="wpool", bufs=1) as wpool,
            tc.tile_pool(name="xpool", bufs=4) as xpool,
            tc.tile_pool(name="hpool", bufs=3) as hpool,
            tc.tile_pool(name="opool", bufs=3) as opool,
            tc.tile_pool(name="ps_h", bufs=PS_H_BUFS if fuse_gelu else 6, space="PSUM") as ps_h_pool,
            tc.tile_pool(name="ps_o", bufs=PS_O_BUFS, space="PSUM") as ps_o_pool,
        ):
            # --- PE warmup: no data deps, runs during the initial DMA wait ---
            if N_WARMUP_MM:
                warm_x = wpool.tile([P, TILE], F16)
                nc.gpsimd.memset(warm_x, 0.0)
                warm_shape = [P, 2, TILE] if fuse_gelu else [P, TILE]
                warm_ps = ps_h_pool.tile(warm_shape, F32, tag="ps_h")
                warm_ps = warm_ps[:, 0] if fuse_gelu else warm_ps
                for _ in range(N_WARMUP_MM):
                    nc.tensor.matmul(warm_ps, warm_x[:, :P], warm_x,
                                     start=True, stop=True)

            # --- Weight/x loads: x0 (256KB) and the first 256KB piece of w1
            # are the critical path for tile 0's first h-chunks; they go on
            # separate queues so both stream at once. Remaining w1 pieces and
            # x1 follow behind.
            w1_sb = wpool.tile([P, HC, KC, P], F16)
            x_sb0 = xpool.tile([P, KC, sizes[0]], F16, tag="x")
            b1_sb = wpool.tile([P, HC], F32)
            b2_sb = wpool.tile([A, 1], F32)
            nc.scalar.dma_start(
                out=x_sb0,
                in_=x_in[:, 0:KC * sizes[0]].rearrange("p (kc n) -> p kc n", kc=KC))
            for piece in range(4):  # 2 h-chunks (256KB) per piece
                h0 = piece * 2
                nc.sync.dma_start(out=w1_sb[:, h0:h0 + 2], in_=w1_in[:, h0:h0 + 2])
            nc.gpsimd.dma_start(out=b1_sb, in_=b1_in[:])
            nc.gpsimd.dma_start(out=b2_sb, in_=b2_in[:])
            w2_sb = wpool.tile([P, HC, A], F16)
            nc.gpsimd.dma_start(out=w2_sb, in_=w2_in[:])
            x_sb1 = None
            if len(sizes) > 1:
                x_sb1 = xpool.tile([P, KC, sizes[1]], F16, tag="x")
                a = KC * sizes[0]
                nc.gpsimd.dma_start(
                    out=x_sb1,
                    in_=x_in[:, a:a + KC * sizes[1]].rearrange("p (kc n) -> p kc n", kc=KC))

            def emit_l2(h_sb, off, size, t, packed):
                """Layer 2: out[:, off:off+size] = W2^T h + b2."""
                o_sb = opool.tile([A, size], F32, tag="o")
                if packed:
                    # 4 h-chunks run concurrently in the 4 PE column groups,
                    # accumulating 2 rounds; strips combined on DVE (which may
                    # read at most one PSUM operand per instruction).
                    o_ps = ps_o_pool.tile([P, size], F32, tag="ps_o")
                    for r in range(2):
                        for j in range(4):
                            hc = r * 4 + j
                            nc.tensor.matmul(
                                o_ps[32 * j:32 * j + A, :],
                                w2_sb[:, hc],
                                h_sb[:, hc],
                                start=(r == 0),
                                stop=(r == 1),
                                tile_position=(0, 32 * j),
                            )
                    nc.vector.tensor_scalar_add(o_sb, o_ps[0:A], b2_sb)
                    nc.vector.tensor_add(o_sb, o_sb, o_ps[32:32 + A])
                    nc.vector.tensor_add(o_sb, o_sb, o_ps[64:64 + A])
                    nc.vector.tensor_add(o_sb, o_sb, o_ps[96:96 + A])
                else:
                    o_ps = ps_o_pool.tile([A, size], F32, tag="ps_o")
                    for hc in range(HC):
                        nc.tensor.matmul(
                            o_ps,
                            w2_sb[:, hc],
                            h_sb[:, hc],
                            start=(hc == 0),
                            stop=(hc == HC - 1),
                        )
                    if fuse_gelu:
                        # b2 == 0: PSUM->SBUF copy on ACT (idle at the tail)
                        # so the store doesn't queue behind the previous
                        # tile's DVE strip-combine on the in-order Vector
                        nc.scalar.activation(o_sb, o_ps,
                                             mybir.ActivationFunctionType.Copy)
                    else:
                        nc.vector.tensor_scalar_add(o_sb, o_ps, b2_sb)
                # alternate store queues so the final two stores issue in
                # parallel instead of serializing on one engine
                eng = nc.sync if t % 2 == 0 else nc.scalar
                eng.dma_start(out=out[:, off:off + size], in_=o_sb)

            # Layer 2 for tile t is emitted mid-way through layer 1 of tile
            # t+1 so its matmuls never wait on a just-finished GELU (PE is
            # in-order) and its DVE/store epilogue drains under compute. The
            # final tile uses unpacked L2: its single-op DVE epilogue keeps
            # the drain tail short.
            pend = None
            off = 0
            for t, size in enumerate(sizes):
                if t == 0:
                    x_sb = x_sb0
                elif t == 1 and x_sb1 is not None:
                    x_sb = x_sb1
                else:
                    x_sb = xpool.tile([P, KC, size], F16, tag="x")
                    a = KC * off
                    nc.sync.dma_start(
                        out=x_sb,
                        in_=x_in[:, a:a + KC * size].rearrange("p (kc n) -> p kc n", kc=KC))

                # --- Layer 1: h = gelu(W1^T x + b1), per 128-row h-chunk ---
                h_sb = hpool.tile([P, HC, size], F16, tag="h")

                def flush_pend(pend=pend):
                    if pend is not None:
                        packed = PACK_L2 and pend[3] < len(sizes) - 1
                        emit_l2(*pend, packed)

                if fuse_gelu:
                    # b1 == 0: one ACTIVATE per pair of h-chunks (2 PSUM
                    # banks) halves ACT per-op overhead; ACT is otherwise
                    # rate-matched with PE and every hiccup stalls it.
                    # The pair tile is allocated at the full TILE width so
                    # each half starts on a 2KB PSUM bank boundary — a
                    # [P, 2, size<512] tile would put half 1 mid-bank and
                    # matmul accumulation breaks across a bank straddle.
                    for hg in range(HC // 2):
                        ps = ps_h_pool.tile([P, 2, TILE], F32, tag="ps_h")
                        for half in range(2):
                            hc = hg * 2 + half
                            for kc in range(KC):
                                nc.tensor.matmul(
                                    ps[:, half, :size],
                                    w1_sb[:, hc, kc],
                                    x_sb[:, kc],
                                    start=(kc == 0),
                                    stop=(kc == KC - 1),
                                )
                        nc.scalar.activation(
                            h_sb[:, hg * 2:hg * 2 + 2], ps[:, :, :size],
                            mybir.ActivationFunctionType.Gelu,
                        )
                        if hg == 0:
                            # previous tile's layer 2 goes here: mid-tile so
                            # its DVE/store epilogue drains before this
                            # tile's L1 ends (shorter pipeline tail)
                            flush_pend()
                else:
                    for hc in range(HC):
                        ps = ps_h_pool.tile([P, size], F32, tag="ps_h")
                        for kc in range(KC):
                            nc.tensor.matmul(
                                ps,
                                w1_sb[:, hc, kc],
                                x_sb[:, kc],
                                start=(kc == 0),
                                stop=(kc == KC - 1),
                            )
                        nc.scalar.activation(
                            h_sb[:, hc], ps,
                            mybir.ActivationFunctionType.Gelu,
                            bias=b1_sb[:, hc:hc + 1],
                        )
                        if hc == 1:
                            flush_pend()

                pend = (h_sb, off, size, t)
                off += size

            packed = PACK_L2 and pend[3] < len(sizes) - 1
            emit_l2(*pend, packed)

    nc.finalize()
    return nc


def kernel(pred_action_latents, W1, b1, W2, b2, embodiment_ids):
    x = np.asarray(pred_action_latents)
    W1 = np.asarray(W1)
    b1 = np.asarray(b1)
    W2 = np.asarray(W2)
    b2 = np.asarray(b2)
    ids = np.asarray(embodiment_ids)

    B, T, _ = x.shape
    assert W1.shape[0] == E and N_CORES == 2 * E

    # --- Host-side routing/sharding ---
    order = np.argsort(ids, kind="stable")
    counts = np.bincount(ids, minlength=E)
    starts = np.concatenate([[0], np.cumsum(counts)])

    # core 2e, 2e+1 handle expert e (first/second half of its rows)
    core_rows = []
    for e in range(E):
        rows_e = order[starts[e]:starts[e + 1]]
        h1 = (len(rows_e) + 1) // 2
        core_rows.append(rows_e[:h1])
        core_rows.append(rows_e[h1:])

    max_tok = max(len(r) * T for r in core_rows)
    ntok = max(GRAIN, ((max_tok + GRAIN - 1) // GRAIN) * GRAIN)

    fuse_gelu = not np.any(b1) and not np.any(b2)
    key = (ntok, fuse_gelu)
    if key not in _PROGRAM_CACHE:
        _PROGRAM_CACHE[key] = _build_program(ntok, fuse_gelu)
    nc = _PROGRAM_CACHE[key]

    in_maps = []
    for c in range(N_CORES):
        e = c // 2
        rows = core_rows[c]
        ntok_real = len(rows) * T
        xr = np.zeros((ntok, D), dtype=np.float32)
        xr[:ntok_real] = x[rows].reshape(ntok_real, D)
        # tile-blocked [P, KC*ntok]: tile block t = [P, KC, size] with
        # (p, kc, n) = xr[off+n, kc*P+p]; 4KB-contiguous per-partition runs
        blocks = []
        o = 0
        for size in _tile_sizes(ntok):
            blocks.append(
                xr[o:o + size].reshape(size, KC, P).transpose(2, 1, 0).reshape(P, KC * size))
            o += size
        x_dev = np.ascontiguousarray(np.concatenate(blocks, axis=1)).astype(np.float16)
        # [P, HC, KC, 128]: (p, hc, kc, j) = W1[e, kc*P+p, hc*P+j]
        w1_dev = np.ascontiguousarray(
            W1[e].reshape(KC, P, HC, P).transpose(1, 2, 0, 3)
        ).astype(np.float16)
        w2_dev = np.ascontiguousarray(
            W2[e].reshape(HC, P, A).transpose(1, 0, 2)
        ).astype(np.float16)
        b1_dev = np.ascontiguousarray(b1[e].reshape(HC, P).T).astype(np.float32)
        b2_dev = np.ascontiguousarray(b2[e].reshape(A, 1)).astype(np.float32)
        in_maps.append({
            "x": x_dev, "w1": w1_dev, "w2": w2_dev, "b1": b1_dev, "b2": b2_dev,
        })

    trace = TRACE_SINK is not None
    if trace:
        os.environ.pop("BASS_NEVER_TRACE", None)
    else:
        # An ambient BASS_TRACE would route run_bass_kernel_spmd through the
        # axon NTFF hook, which needs antenv.axon_hooks (absent in fresh
        # containers) — force tracing off unless explicitly requested.
        os.environ["BASS_NEVER_TRACE"] = "1"
    res = run_bass_kernel_spmd(nc, in_maps, core_ids=list(range(N_CORES)),
                               trace=trace)
    if trace:
        TRACE_SINK["exec_time_ns"] = res.exec_time_ns
        TRACE_SINK["mean_exec_time_ns"] = res.mean_exec_time_ns
        TRACE_SINK["profile_json"] = res.profile_json

    # --- Host-side unshard ---
    out_full = np.zeros((B, T, A), dtype=np.float32)
    for c in range(N_CORES):
        rows = core_rows[c]
        if len(rows) == 0:
            continue
        o = np.asarray(res.results[c]["out"])  # [A, ntok] f32
        out_full[rows] = o[:, :len(rows) * T].T.reshape(len(rows), T, A)
    return out_full


# revision 8
# speedup vs baseline: 1.3993x; 1.0137x over previous
"""Trainium2 Bass kernel for nn_ActionDecoder (MoE-routed 2-layer GELU MLP).

Problem: per batch row b (2048 rows x 16 timesteps), route through the
embodiment_ids[b]-th expert MLP: out = GELU(x @ W1[e] + b1[e]) @ W2[e] + b2[e].
x: [2048, 16, 512] f32, W1: [4, 512, 1024], W2: [4, 1024, 28].

Strategy (expert-parallel): host sorts batch rows by embodiment, gives each of
the 8 cores one expert (2 cores per expert, half the expert's rows each). Each
core runs a dense 2-layer MLP over its tokens with its own expert's weights
(weights are per-core *data*, so one SPMD program serves all cores). Activations
are fed transposed ([d, tok]) so both matmuls keep weights stationary; compute
in fp16 (same PE rate as bf16, 8x finer mantissa) with fp32 PSUM accumulation.
(fp8 was evaluated and rejected: e4m3 DoubleRow runs at the same column rate
as fp16 so accuracy-preserving hi/lo splits cancel the MAC gain, and
single-fp8 error (>2e-2 rel even with optimal linear corrections) fails the
gate; e3m4 DoubleRow is rejected by walrus codegen.)

Perf notes:
- Token dim tiled as one 256-token lead tile (so the first matmuls wait on
  only 256 KB of x), then 512-token tiles, then a 256..767 remainder; every
  size >= 256 keeps the PE at its 216 ns/matmul streaming rate.
- W1 is DMA'd in four 256 KB pieces so tile 0's first h-chunks start after
  one piece lands instead of after the full 1 MB.
- Layer 2 (M=28) packs 4 h-chunks into the 4 PE column groups concurrently
  (tile_position), then combines the 4 PSUM partition strips on DVE.
- A few dependency-free warmup matmuls run during the initial DMA wait to
  lift the PE HAM clock gate to 8/8 before real work arrives.
"""

import os

import numpy as np

import concourse.bacc as bacc
import concourse.mybir as mybir
from concourse.tile import TileContext
from concourse.bass_utils import run_bass_kernel_spmd

# Model dims (hardcoded per problem spec)
D = 512      # d_model
H = 1024     # hidden
A = 28       # max action dim
E = 4        # n embodiments
N_CORES = 8
P = 128      # partitions
TILE = 512   # main token tile
LEAD = 256   # first tile: small so compute starts on 256KB of x
GRAIN = 128  # token granularity (min tile)
KC = D // P  # 4 contraction chunks for layer 1
HC = H // P  # 8 hidden chunks

PS_H_BUFS = 3      # fused-gelu L1 PSUM slots (2 banks each)
PS_O_BUFS = 2      # layer-2 PSUM slots (1 bank each); ps_h*2 + ps_o <= 8
N_WARMUP_MM = 7    # spans the PE p-state ramp (~3us of continuous execution)
PACK_L2 = True     # pack layer-2 into PE column groups

F32 = mybir.dt.float32
F16 = mybir.dt.float16

_PROGRAM_CACHE = {}

# Set by test harness to collect a profile: None | dict (filled with results)
TRACE_SINK = None


def _tile_sizes(ntok):
    if ntok <= TILE + LEAD:
        # too small to split a lead tile off; mirror the simple scheme
        sizes = [TILE] * (ntok // TILE)
        if ntok % TILE:
            sizes.append(ntok % TILE)
        return sizes
    rest = ntok - LEAD
    n_full = (rest - LEAD) // TILE  # keep remainder >= LEAD
    rem = rest - n_full * TILE      # in [LEAD, TILE+LEAD)
    return [LEAD] + [TILE] * n_full + [rem]


def _build_program(ntok, fuse_gelu):
    assert ntok % GRAIN == 0
    sizes = _tile_sizes(ntok)
    nc = bacc.Bacc()

    # x is tile-blocked: tile t occupies columns [KC*off, KC*(off+size)) as
    # a [KC, size] block, so every DMA reads 4KB-contiguous per-partition runs
    x_in = nc.declare_dram_parameter("x", [P, KC * ntok], F16, isOutput=False)
    w1_in = nc.declare_dram_parameter("w1", [P, HC, KC, P], F16, isOutput=False)
    w2_in = nc.declare_dram_parameter("w2", [P, HC, A], F16, isOutput=False)
    b1_in = nc.declare_dram_parameter("b1", [P, HC], F32, isOutput=False)
    b2_in = nc.declare_dram_parameter("b2", [A, 1], F32, isOutput=False)
    out = nc.declare_dram_parameter("out", [A, ntok], F32, isOutput=True)

    with TileContext(nc) as tc:
        with (
            tc.tile_pool(name="wpool", bufs=1) as wpool,
            tc.tile_pool(name="xpool", bufs=4) as xpool,
            tc.tile_pool(name="hpool", bufs=3) as hpool,
            tc.tile_pool(name="opool", bufs=3) as opool,
            tc.tile_pool(name="ps_h", bufs=PS_H_BUFS if fuse_gelu else 6, space="PSUM") as ps_h_pool,
            tc.tile_pool(name="ps_o", bufs=PS_O_BUFS, space="PSUM") as ps_o_pool,
        ):
            # --- PE warmup: no data deps, runs during the initial DMA wait.
            # memset on DVE (otherwise idle until the first L2 combine) so
            # the first warmup launches as soon as the PE preamble ends.
            if N_WARMUP_MM:
                warm_x = wpool.tile([P, TILE], F16)
                nc.vector.memset(warm_x, 0.0)
                warm_shape = [P, 2, TILE] if fuse_gelu else [P, TILE]
                warm_ps = ps_h_pool.tile(warm_shape, F32, tag="ps_h")
                warm_ps = warm_ps[:, 0] if fuse_gelu else warm_ps
                for _ in range(N_WARMUP_MM):
                    nc.tensor.matmul(warm_ps, warm_x[:, :P], warm_x,
                                     start=True, stop=True)

            # --- Weight/x loads: x0 (256KB) and the first 256KB piece of w1
            # are the critical path for tile 0's first h-chunks; they go on
            # separate queues so both stream at once. Remaining w1 pieces and
            # x1 follow behind.
            w1_sb = wpool.tile([P, HC, KC, P], F16)
            x_sb0 = xpool.tile([P, KC, sizes[0]], F16, tag="x")
            b1_sb = wpool.tile([P, HC], F32)
            b2_sb = wpool.tile([A, 1], F32)
            nc.scalar.dma_start(
                out=x_sb0,
                in_=x_in[:, 0:KC * sizes[0]].rearrange("p (kc n) -> p kc n", kc=KC))
            nc.sync.dma_start(out=w1_sb[:, 0:HC // 2], in_=w1_in[:, 0:HC // 2])
            nc.sync.dma_start(out=w1_sb[:, HC // 2:], in_=w1_in[:, HC // 2:])
            w2_sb = wpool.tile([P, HC, A], F16)
            x_sb1 = None
            if len(sizes) > 1:
                # x1 leads the gpsimd queue: its 512KB must be in flight
                # early so tile 1 doesn't stall behind the small misc loads
                x_sb1 = xpool.tile([P, KC, sizes[1]], F16, tag="x")
                a = KC * sizes[0]
                nc.gpsimd.dma_start(
                    out=x_sb1,
                    in_=x_in[:, a:a + KC * sizes[1]].rearrange("p (kc n) -> p kc n", kc=KC))
            nc.gpsimd.dma_start(out=w2_sb, in_=w2_in[:])
            nc.gpsimd.dma_start(out=b1_sb, in_=b1_in[:])
            nc.gpsimd.dma_start(out=b2_sb, in_=b2_in[:])

            def emit_l2(h_sb, off, size, t, packed):
                """Layer 2: out[:, off:off+size] = W2^T h + b2."""
                o_sb = opool.tile([A, size], F32, tag="o")
                if packed:
                    # 4 h-chunks run concurrently in the 4 PE column groups,
                    # accumulating 2 rounds; strips combined on DVE (which may
                    # read at most one PSUM operand per instruction).
                    o_ps = ps_o_pool.tile([P, size], F32, tag="ps_o")
                    for r in range(2):
                        for j in range(4):
                            hc = r * 4 + j
                            nc.tensor.matmul(
                                o_ps[32 * j:32 * j + A, :],
                                w2_sb[:, hc],
                                h_sb[:, hc],
                                start=(r == 0),
                                stop=(r == 1),
                                tile_position=(0, 32 * j),
                            )
                    nc.vector.tensor_scalar_add(o_sb, o_ps[0:A], b2_sb)
                    nc.vector.tensor_add(o_sb, o_sb, o_ps[32:32 + A])
                    nc.vector.tensor_add(o_sb, o_sb, o_ps[64:64 + A])
                    nc.vector.tensor_add(o_sb, o_sb, o_ps[96:96 + A])
                else:
                    o_ps = ps_o_pool.tile([A, size], F32, tag="ps_o")
                    for hc in range(HC):
                        nc.tensor.matmul(
                            o_ps,
                            w2_sb[:, hc],
                            h_sb[:, hc],
                            start=(hc == 0),
                            stop=(hc == HC - 1),
                        )
                    if fuse_gelu:
                        # b2 == 0: PSUM->SBUF copy on ACT (idle at the tail)
                        # so the store doesn't queue behind the previous
                        # tile's DVE strip-combine on the in-order Vector
                        nc.scalar.activation(o_sb, o_ps,
                                             mybir.ActivationFunctionType.Copy)
                    else:
                        nc.vector.tensor_scalar_add(o_sb, o_ps, b2_sb)
                # alternate store queues so the final two stores issue in
                # parallel instead of serializing on one engine
                eng = nc.sync if t % 2 == 0 else nc.scalar
                eng.dma_start(out=out[:, off:off + size], in_=o_sb)

            # Layer 2 for tile t is emitted mid-way through layer 1 of tile
            # t+1 so its matmuls never wait on a just-finished GELU (PE is
            # in-order) and its DVE/store epilogue drains under compute. The
            # final tile uses unpacked L2: its single-op DVE epilogue keeps
            # the drain tail short.
            pend = None
            off = 0
            for t, size in enumerate(sizes):
                if t == 0:
                    x_sb = x_sb0
                elif t == 1 and x_sb1 is not None:
                    x_sb = x_sb1
                else:
                    x_sb = xpool.tile([P, KC, size], F16, tag="x")
                    a = KC * off
                    nc.sync.dma_start(
                        out=x_sb,
                        in_=x_in[:, a:a + KC * size].rearrange("p (kc n) -> p kc n", kc=KC))

                # --- Layer 1: h = gelu(W1^T x + b1), per 128-row h-chunk ---
                h_sb = hpool.tile([P, HC, size], F16, tag="h")

                def flush_pend(pend=pend):
                    if pend is not None:
                        packed = PACK_L2 and pend[3] < len(sizes) - 1
                        emit_l2(*pend, packed)

                if fuse_gelu:
                    # b1 == 0: one ACTIVATE per pair of h-chunks (2 PSUM
                    # banks) halves ACT per-op overhead; ACT is otherwise
                    # rate-matched with PE and every hiccup stalls it.
                    # The pair tile is allocated at the full TILE width so
                    # each half starts on a 2KB PSUM bank boundary — a
                    # [P, 2, size<512] tile would put half 1 mid-bank and
                    # matmul accumulation breaks across a bank straddle.
                    for hg in range(HC // 2):
                        ps = ps_h_pool.tile([P, 2, TILE], F32, tag="ps_h")
                        for half in range(2):
                            hc = hg * 2 + half
                            for kc in range(KC):
                                nc.tensor.matmul(
                                    ps[:, half, :size],
                                    w1_sb[:, hc, kc],
                                    x_sb[:, kc],
                                    start=(kc == 0),
                                    stop=(kc == KC - 1),
                                )
                        nc.scalar.activation(
                            h_sb[:, hg * 2:hg * 2 + 2], ps[:, :, :size],
                            mybir.ActivationFunctionType.Gelu,
                        )
                        if hg == 0:
                            # previous tile's layer 2 goes here: mid-tile so
                            # its DVE/store epilogue drains before this
                            # tile's L1 ends (shorter pipeline tail)
                            flush_pend()
                else:
                    for hc in range(HC):
                        ps = ps_h_pool.tile([P, size], F32, tag="ps_h")
                        for kc in range(KC):
                            nc.tensor.matmul(
                                ps,
                                w1_sb[:, hc, kc],
                                x_sb[:, kc],
                                start=(kc == 0),
                                stop=(kc == KC - 1),
                            )
                        nc.scalar.activation(
                            h_sb[:, hc], ps,
                            mybir.ActivationFunctionType.Gelu,
                            bias=b1_sb[:, hc:hc + 1],
                        )
                        if hc == 1:
                            flush_pend()

                pend = (h_sb, off, size, t)
                off += size

            packed = PACK_L2 and pend[3] < len(sizes) - 1
            emit_l2(*pend, packed)

    nc.finalize()
    return nc


def kernel(pred_action_latents, W1, b1, W2, b2, embodiment_ids):
    x = np.asarray(pred_action_latents)
    W1 = np.asarray(W1)
    b1 = np.asarray(b1)
    W2 = np.asarray(W2)
    b2 = np.asarray(b2)
    ids = np.asarray(embodiment_ids)

    B, T, _ = x.shape
    assert W1.shape[0] == E and N_CORES == 2 * E

    # --- Host-side routing/sharding ---
    order = np.argsort(ids, kind="stable")
    counts = np.bincount(ids, minlength=E)
    starts = np.concatenate([[0], np.cumsum(counts)])

    # core 2e, 2e+1 handle expert e (first/second half of its rows)
    core_rows = []
    for e in range(E):
        rows_e = order[starts[e]:starts[e + 1]]
        h1 = (len(rows_e) + 1) // 2
        core_rows.append(rows_e[:h1])
        core_rows.append(rows_e[h1:])

    max_tok = max(len(r) * T for r in core_rows)
    ntok = max(GRAIN, ((max_tok + GRAIN - 1) // GRAIN) * GRAIN)

    fuse_gelu = not np.any(b1) and not np.any(b2)
    key = (ntok, fuse_gelu)
    if key not in _PROGRAM_CACHE:
        _PROGRAM_CACHE[key] = _build_program(ntok, fuse_gelu)
    nc = _PROGRAM_CACHE[key]

    in_maps = []
    for c in range(N_CORES):
        e = c // 2
        rows = core_rows[c]
        ntok_real = len(rows) * T
        xr = np.zeros((ntok, D), dtype=np.float32)
        xr[:ntok_real] = x[rows].reshape(ntok_real, D)
        # tile-blocked [P, KC*ntok]: tile block t = [P, KC, size] with
        # (p, kc, n) = xr[off+n, kc*P+p]; 4KB-contiguous per-partition runs
        blocks = []
        o = 0
        for size in _tile_sizes(ntok):
            blocks.append(
                xr[o:o + size].reshape(size, KC, P).transpose(2, 1, 0).reshape(P, KC * size))
            o += size
        x_dev = np.ascontiguousarray(np.concatenate(blocks, axis=1)).astype(np.float16)
        # [P, HC, KC, 128]: (p, hc, kc, j) = W1[e, kc*P+p, hc*P+j]
        w1_dev = np.ascontiguousarray(
            W1[e].reshape(KC, P, HC, P).transpose(1, 2, 0, 3)
        ).astype(np.float16)
        w2_dev = np.ascontiguousarray(
            W2[e].reshape(HC, P, A).transpose(1, 0, 2)
        ).astype(np.float16)
        b1_dev = np.ascontiguousarray(b1[e].reshape(HC, P).T).astype(np.float32)
        b2_dev = np.ascontiguousarray(b2[e].reshape(A, 1)).astype(np.float32)
        in_maps.append({
            "x": x_dev, "w1": w1_dev, "w2": w2_dev, "b1": b1_dev, "b2": b2_dev,
        })

    trace = TRACE_SINK is not None
    if trace:
        os.environ.pop("BASS_NEVER_TRACE", None)
    else:
        # An ambient BASS_TRACE would route run_bass_kernel_spmd through the
        # axon NTFF hook, which needs antenv.axon_hooks (absent in fresh
        # containers) — force tracing off unless explicitly requested.
        os.environ["BASS_NEVER_TRACE"] = "1"
    res = run_bass_kernel_spmd(nc, in_maps, core_ids=list(range(N_CORES)),
                               trace=trace)
    if trace:
        TRACE_SINK["exec_time_ns"] = res.exec_time_ns
        TRACE_SINK["mean_exec_time_ns"] = res.mean_exec_time_ns
        TRACE_SINK["profile_json"] = res.profile_json

    # --- Host-side unshard ---
    out_full = np.zeros((B, T, A), dtype=np.float32)
    for c in range(N_CORES):
        rows = core_rows[c]
        if len(rows) == 0:
            continue
        o = np.asarray(res.results[c]["out"])  # [A, ntok] f32
        out_full[rows] = o[:, :len(rows) * T].T.reshape(len(rows), T, A)
    return out_full
